# revision 1
# baseline (speedup 1.0000x reference)
"""Trainium2 Bass kernel for nn_E30DiagonalGatedCell (T=2048, B=16, D=1024).

Strategy (8 NeuronCores, batch-sharded SPMD — no cross-core communication):
  - Batch dim B=16 is split 2-per-core; weights are replicated.
  - All tensors live in a transposed, e-chunked layout on device so that the
    sequential tanh recurrence keeps the 128-partition dim fully busy:
    the per-step matmul uses W_hT tiles [128d x 128e] as the PE stationary
    operand and the transposed hidden state hT [128, 2] as the moving
    operand, accumulating all 8 d-chunks into one PSUM bank; the output is
    already transposed for the next step (no per-step transpose).
  - W_h, W_x, x, and the h state ring are fp16 (PSUM accumulation fp32):
    LDWEIGHTS bandwidth doubles; recurrence error stays ~1e-3 (tanh is
    contractive, spectral radius ~0.9, so fp16 noise does not compound).
  - xp = x@W_x.T + b is precomputed in one batched GEMM to DRAM scratch.
  - The gate (silu) and outputs are computed per 32-step block, fused into
    the recurrence loop; host only does layout transforms (shard/unshard).
"""
import sys
for p in ('/opt/trn_rl_repo', '/root/.axon_site/_ro/trn_rl_repo'):
    if p not in sys.path:
        sys.path.insert(0, p)
import numpy as np

T, B, D = 2048, 16, 1024
NCORES = 8
Bc = B // NCORES
NCHUNK = 8
UNROLL = 32

# ---------------------------------------------------------------- tile patches


def _apply_tile_patches():
    """Workarounds for the pinned walrus build: it encodes at most one
    sem-wait per instruction, so split Tile's multi-wait drains/instructions
    onto single-wait EventSemaphore carriers."""
    from concourse import tile
    from concourse.vector_clock import ScopedClock, VectorClock

    def _drain_and_barrier_split(self, tick_clock, wait_clock):
        gc = tick_clock.global_clock
        n = len(gc)
        for proc in range(n):
            t = gc[proc]
            if t <= 0:
                continue
            vc = VectorClock([0] * n)
            vc.require_at_least(proc, t)
            drain_inst = self.nc.sync.drain()
            wait_clock.add_sem_waits(drain_inst.ins, ScopedClock({None: vc}))
        self.nc.all_engine_barrier()
        assert self.sems is not None
        popped = self.nc._tile_sem_poison_stack.pop()
        assert popped is self._sem_poison
        self.nc.clear_and_free_semaphores(list(self.sems.allocated().values()))
        self.nc.all_engine_barrier()

    tile.TileContext._drain_and_barrier = _drain_and_barrier_split


def _split_sync_waits(nc, maxw=1):
    from concourse import mybir
    for fn in nc.m.functions:
        for bb in fn.blocks:
            out = []
            for inst in bb.instructions:
                si = inst.sync_info
                if si is not None and si.on_wait and len(si.on_wait) > maxw:
                    waits = list(si.on_wait)
                    extra, keep = waits[:-maxw], waits[-maxw:]
                    for k, w in enumerate(extra):
                        out.append(mybir.InstEventSemaphore(
                            name=f"{inst.name}-wsplit{k}",
                            engine=inst.engine, ins=[], outs=[],
                            sync_info=mybir.SyncInfo(on_wait=[w], on_update=[]),
                        ))
                    inst.sync_info = mybir.SyncInfo(on_wait=keep,
                                                    on_update=list(si.on_update))
                out.append(inst)
            bb.instructions[:] = out


# ---------------------------------------------------------------- bass program

def _build_program():
    _apply_tile_patches()
    import concourse.bass as bass
    import concourse.mybir as mybir
    from concourse import tile

    F32 = mybir.dt.float32
    F16 = mybir.dt.float16
    AF = mybir.ActivationFunctionType
    ALU = mybir.AluOpType
    TB = T * Bc
    NT = 512
    BLK = UNROLL * Bc
    CB = NCHUNK * Bc

    nc = bass.Bass()
    xT = nc.declare_dram_parameter("xT", [D, TB], F16, isOutput=False)
    zTr = nc.declare_dram_parameter("zTr", [128, NCHUNK, TB], F32, isOutput=False)
    whT_e = nc.declare_dram_parameter("whT", [D, D], F16, isOutput=False)
    wxT_e = nc.declare_dram_parameter("wxT", [D, D], F16, isOutput=False)
    h0T = nc.declare_dram_parameter("h0T", [128, NCHUNK * Bc], F16, isOutput=False)
    bias_e = nc.declare_dram_parameter("bias", [128, NCHUNK], F32, isOutput=False)
    gz_e = nc.declare_dram_parameter("gz", [128, NCHUNK], F32, isOutput=False)
    gh_e = nc.declare_dram_parameter("gh", [128, NCHUNK], F32, isOutput=False)
    bg_e = nc.declare_dram_parameter("bg", [128, NCHUNK], F32, isOutput=False)
    houtT = nc.declare_dram_parameter("houtT", [128, NCHUNK, TB], F16, isOutput=True)
    outT = nc.declare_dram_parameter("outT", [128, NCHUNK, TB], F32, isOutput=True)
    xp_scr = nc.dram_tensor("xp_scr", [128, NCHUNK, TB], F32)

    with tile.TileContext(nc) as tc:
        with (
            tc.tile_pool(name="wpool", bufs=1) as wpool,
            tc.tile_pool(name="iopool", bufs=2) as iopool,
            tc.tile_pool(name="tpool", bufs=4) as tpool,
            tc.tile_pool(name="psum", bufs=2, space="PSUM") as pgemm,
            tc.tile_pool(name="psum_r", bufs=6, space="PSUM") as prec,
        ):
            whT = wpool.tile([128, NCHUNK, D], F16, tag="whT")
            wxT = wpool.tile([128, NCHUNK, D], F16, tag="wxT")
            H = wpool.tile([128, NCHUNK, BLK], F16, tag="H")
            XP = wpool.tile([128, NCHUNK, BLK], F32, tag="XP")
            OUTS = wpool.tile([128, NCHUNK, BLK], F32, tag="OUTS")
            ZB = wpool.tile([128, NCHUNK, BLK], F32, tag="ZB")
            vbias = wpool.tile([128, NCHUNK], F32, tag="vbias")
            vgz = wpool.tile([128, NCHUNK], F32, tag="vgz")
            vgh = wpool.tile([128, NCHUNK], F32, tag="vgh")
            vbg = wpool.tile([128, NCHUNK], F32, tag="vbg")

            for k in range(NCHUNK):
                nc.sync.dma_start(whT[:, k, :], whT_e[k * 128:(k + 1) * 128, :])
                nc.sync.dma_start(wxT[:, k, :], wxT_e[k * 128:(k + 1) * 128, :])
            nc.sync.dma_start(vbias[:], bias_e[:])
            nc.sync.dma_start(vgz[:], gz_e[:])
            nc.sync.dma_start(vgh[:], gh_e[:])
            nc.sync.dma_start(vbg[:], bg_e[:])
            for c in range(NCHUNK):
                nc.sync.dma_start(H[:, c, (UNROLL - 1) * Bc:UNROLL * Bc],
                                  h0T[:, c * Bc:(c + 1) * Bc])

            # phase 1: xp = x @ W_x.T + b
            for n in range(TB // NT):
                xn = iopool.tile([128, NCHUNK, NT], F16, tag="xn")
                for k in range(NCHUNK):
                    nc.sync.dma_start(xn[:, k, :],
                                      xT[k * 128:(k + 1) * 128, n * NT:(n + 1) * NT])
                for c in range(NCHUNK):
                    ps = pgemm.tile([128, NT], F32, tag="psg")
                    for k in range(NCHUNK):
                        nc.tensor.matmul(ps[:], wxT[:, k, c * 128:(c + 1) * 128],
                                         xn[:, k, :],
                                         start=(k == 0), stop=(k == NCHUNK - 1))
                    sb = iopool.tile([128, NT], F32, tag="xpsb")
                    nc.scalar.activation(sb[:], ps[:], AF.Identity,
                                         bias=vbias[:, c:c + 1])
                    nc.sync.dma_start(xp_scr[:, c, n * NT:(n + 1) * NT], sb[:])

            # phase 2: recurrence + gate
            with tc.For_i(0, TB, BLK, hint_engines=(mybir.EngineType.PE,)) as i:
                nc.sync.dma_start(XP[:], xp_scr[:, :, bass.ds(i, BLK)])
                nc.sync.dma_start(ZB[:], zTr[:, :, bass.ds(i, BLK)])

                for j in range(UNROLL):
                    pj = (j - 1) % UNROLL
                    ps = prec.tile([128, CB], F32, tag="psr")
                    for c in range(NCHUNK):
                        for k in range(NCHUNK):
                            nc.tensor.matmul(ps[:, c * Bc:(c + 1) * Bc],
                                             whT[:, k, c * 128:(c + 1) * 128],
                                             H[:, k, pj * Bc:(pj + 1) * Bc],
                                             start=(k == 0), stop=(k == NCHUNK - 1))
                    tt = tpool.tile([128, CB], F32, tag="tt")
                    nc.vector.tensor_add(tt[:], ps[:], XP[:, :, j * Bc:(j + 1) * Bc])
                    nc.scalar.activation(H[:, :, j * Bc:(j + 1) * Bc], tt[:], AF.Tanh)

                for c in range(NCHUNK):
                    hf = tpool.tile([128, BLK], F32, tag="hf")
                    nc.vector.tensor_copy(hf[:], H[:, c, :])
                    g1 = tpool.tile([128, BLK], F32, tag="g1")
                    nc.vector.tensor_scalar(g1[:], ZB[:, c, :], vgz[:, c:c + 1],
                                            vbg[:, c:c + 1], op0=ALU.mult, op1=ALU.add)
                    g2 = tpool.tile([128, BLK], F32, tag="g2")
                    nc.vector.scalar_tensor_tensor(g2[:], hf[:], vgh[:, c:c + 1], g1[:],
                                                   op0=ALU.mult, op1=ALU.add)
                    g3 = tpool.tile([128, BLK], F32, tag="g3")
                    nc.scalar.activation(g3[:], g2[:], AF.Sigmoid)
                    g4 = tpool.tile([128, BLK], F32, tag="g4")
                    nc.vector.tensor_mul(g4[:], g2[:], g3[:])
                    nc.vector.tensor_mul(OUTS[:, c, :], hf[:], g4[:])

                nc.sync.dma_start(outT[:, :, bass.ds(i, BLK)], OUTS[:])
                nc.sync.dma_start(houtT[:, :, bass.ds(i, BLK)], H[:])
    _split_sync_waits(nc)
    return nc


# ---------------------------------------------------------------- host shims

def _host_prep(x, z, h0, W_x, W_h, b, g_z, g_h, b_gate):
    whT = np.ascontiguousarray(W_h.T.astype(np.float16))
    wxT = np.ascontiguousarray(W_x.T.astype(np.float16))
    vb = np.ascontiguousarray(b.reshape(NCHUNK, 128).T.astype(np.float32))
    vgz = np.ascontiguousarray(g_z.reshape(NCHUNK, 128).T.astype(np.float32))
    vgh = np.ascontiguousarray(g_h.reshape(NCHUNK, 128).T.astype(np.float32))
    vbg = np.ascontiguousarray(b_gate.reshape(NCHUNK, 128).T.astype(np.float32))
    maps = []
    for i in range(NCORES):
        bs = slice(i * Bc, (i + 1) * Bc)
        xTs = np.ascontiguousarray(
            x[:, bs, :].transpose(2, 0, 1).reshape(D, T * Bc).astype(np.float16))
        zTs = np.ascontiguousarray(
            z[:, bs, :].transpose(2, 0, 1).reshape(NCHUNK, 128, T * Bc).transpose(1, 0, 2))
        h0s = np.ascontiguousarray(
            h0[bs].T.reshape(NCHUNK, 128, Bc).transpose(1, 0, 2).reshape(128, NCHUNK * Bc)
        ).astype(np.float16)
        maps.append({"xT": xTs, "zTr": zTs, "whT": whT, "wxT": wxT, "h0T": h0s,
                     "bias": vb, "gz": vgz, "gh": vgh, "bg": vbg})
    return maps


def _host_post(results, h0):
    out = np.empty((T, B, D), np.float32)
    h = np.empty((T + 1, B, D), np.float32)
    h[0] = h0
    for i, r in enumerate(results):
        bs = slice(i * Bc, (i + 1) * Bc)
        ht = r["houtT"].astype(np.float32).reshape(128, NCHUNK, T, Bc)
        h[1:, bs, :] = ht.transpose(2, 3, 1, 0).reshape(T, Bc, D)
        ot = r["outT"].reshape(128, NCHUNK, T, Bc)
        out[:, bs, :] = ot.transpose(2, 3, 1, 0).reshape(T, Bc, D)
    return out, h


_CACHE = {}


def _get_runner():
    if "runner" in _CACHE:
        return _CACHE["runner"]
    nc = _build_program()
    from concourse.bass_utils import run_bass_kernel_spmd
    _CACHE["runner"] = (nc, run_bass_kernel_spmd)
    return _CACHE["runner"]


def kernel(x, z, h0, W_x, W_h, b, g_z, g_h, b_gate):
    x = np.asarray(x, np.float32)
    z = np.asarray(z, np.float32)
    h0 = np.asarray(h0, np.float32)
    maps = _host_prep(x, z, np.asarray(h0), np.asarray(W_x, np.float32),
                      np.asarray(W_h, np.float32), np.asarray(b, np.float32),
                      np.asarray(g_z, np.float32), np.asarray(g_h, np.float32),
                      np.asarray(b_gate, np.float32))
    nc, run = _get_runner()
    res = run(nc, maps, list(range(NCORES)))
    out, h = _host_post(res.results, h0)
    return out, h
